# revision 3
# baseline (speedup 1.0000x reference)
"""DLRM (embedding gather + bottom MLP + pairwise interactions + top MLP)
on 8 Trainium2 NeuronCores, data-parallel over the batch.

Sharding: batch 16384 -> 8 cores x 2048 samples. The embedding table
(26 x 100000 x 128, stored bf16, 0.67 GB) is replicated to every core's HBM,
so no collectives are needed; each core runs the full model on its slice.

v2 changes vs the 441us baseline:
  - emb table stored bf16: gather descriptors 256B (same DMA time as 512B
    fp32 under the sub-512B RMW penalty, half HBM bytes), PE transposes run
    at 1 cycle/row instead of 2 (fp32), no separate cast.
  - gather uses 104 partitions (4 samples x 26 features) - no dummy rows.
  - S -> X1 relayout stays on-chip: diagonal-select PSUM->SBUF copies into
    spk2 [108, 32*27], then ONE SBUF->SBUF DMA per 128-sample block
    rearranges to x1s [128 samples, 729 pairs]; PE transposes build
    X1T [729, batch] tiles. (Baseline round-tripped DRAM with 108B-run
    scatter descriptors - the dominant DMA cost.)
  - top MLP in fp8e4m3 with DoubleRow matmuls (2 K-tiles per instruction):
    X1 is cast fp8 with a x128 scale, weights host-scaled into fp8's normal
    range; each layer's activation applies the inverse scale on fp32 PSUM.
  - interactions + bottom MLP stay bf16 (fp8 without DoubleRow is not
    faster for K=128, and this keeps accuracy headroom).
  - top-MLP btile is interleaved into the block loop (after each 4th block)
    so PE never waits on a phase barrier and SBUF tiles ring-buffer cleanly.
"""

import numpy as np
import ml_dtypes

import concourse.bass as bass
import concourse.mybir as mybir
import concourse.tile as tile
from concourse import bacc
from concourse.bass import ds, ts

F32 = mybir.dt.float32
BF16 = mybir.dt.bfloat16
FP8 = mybir.dt.float8e4
I32 = mybir.dt.int32

N_CAT = 26
N_FEAT = 27          # 26 embeddings + dense
E = 128
P = 128
N_CORES = 8

AFT = mybir.ActivationFunctionType
DR = mybir.MatmulPerfMode.DoubleRow

# fp8 scale plan (host folds these into weights; activations invert on PSUM):
S_SCALE = 128.0      # x1 fp8 = 128 * S
D_SCALE = 4.0        # dense8 fp8 = 4 * dense
W1_SCALE = 16.0      # w1f8 = 16 * W1fold      -> psum1 = 2048 * logit1
W_SCALE = 8.0        # L2..L5 weights x8       -> psum  = 8 * logit


class Cfg:
    def __init__(self, vocab=100000, nblk=16):
        self.vocab = vocab
        self.nblk = nblk                  # 128-sample blocks per core
        self.bc = nblk * P                # batch per core
        self.vblk = min(4, nblk)          # blocks per btile
        self.vt = self.vblk * P           # btile size (free dim N)
        self.nv = nblk // self.vblk       # btiles per core


def build_kernel(nc: bass.Bass, tc: tile.TileContext, cfg: Cfg):
    from contextlib import ExitStack
    with ExitStack() as ctx:
        _build_kernel(nc, tc, cfg, ctx)


def _build_kernel(nc: bass.Bass, tc: tile.TileContext, cfg: Cfg, ctx):
    NBLK, BC, VT, NV, VBLK = cfg.nblk, cfg.bc, cfg.vt, cfg.nv, cfg.vblk

    # ---------------- DRAM I/O ----------------
    emb = nc.dram_tensor("emb", [N_CAT * cfg.vocab, E], BF16, kind="ExternalInput").ap()
    idx_d = nc.dram_tensor("idx", [104, NBLK * 32], I32, kind="ExternalInput").ap()
    xt_d = nc.dram_tensor("xt", [13, BC], BF16, kind="ExternalInput").ap()

    wd1_d = nc.dram_tensor("wd1", [13, 512], BF16, kind="ExternalInput").ap()
    wd2_d = nc.dram_tensor("wd2", [512, 256], BF16, kind="ExternalInput").ap()
    wdf_d = nc.dram_tensor("wdf", [256, 128], BF16, kind="ExternalInput").ap()

    # fp8 top-MLP weights, DoubleRow layout [128, K/128, M]
    w1f8_d = nc.dram_tensor("w1f8", [P, 6 * 1024], FP8, kind="ExternalInput").ap()
    w1d8_d = nc.dram_tensor("w1d8", [P, 1024], FP8, kind="ExternalInput").ap()
    wt28_d = nc.dram_tensor("wt28", [P, 8 * 1024], FP8, kind="ExternalInput").ap()
    wt38_d = nc.dram_tensor("wt38", [P, 8 * 512], FP8, kind="ExternalInput").ap()
    wt48_d = nc.dram_tensor("wt48", [P, 4 * 256], FP8, kind="ExternalInput").ap()
    wo8_d = nc.dram_tensor("wo8", [P, 2 * 1], FP8, kind="ExternalInput").ap()

    def bin_(name, shape):
        return nc.dram_tensor(name, shape, F32, kind="ExternalInput").ap()

    bd1_d = bin_("bd1", [P, 4])
    bd2_d = bin_("bd2", [P, 2])
    bdf_d = bin_("bdf", [P, 1])
    bdf4_d = bin_("bdf4", [P, 1])
    bt1_d = bin_("bt1", [P, 8])
    bt2_d = bin_("bt2", [P, 8])
    bt3_d = bin_("bt3", [P, 4])
    bt4_d = bin_("bt4", [P, 2])
    bo_d = bin_("bo", [1, 1])
    ident_d = nc.dram_tensor("ident", [P, P], BF16, kind="ExternalInput").ap()

    out_d = nc.dram_tensor("out", [BC, 1], F32, kind="ExternalOutput").ap()

    # ---------------- pools ----------------
    const = ctx.enter_context(tc.tile_pool(name="const", bufs=1))
    gpool = ctx.enter_context(tc.tile_pool(name="gather", bufs=2))
    zpool = ctx.enter_context(tc.tile_pool(name="zt", bufs=2))
    spool = ctx.enter_context(tc.tile_pool(name="spack", bufs=2))
    x1pool = ctx.enter_context(tc.tile_pool(name="x1", bufs=2))
    xtpool = ctx.enter_context(tc.tile_pool(name="x1t", bufs=2))
    opool = ctx.enter_context(tc.tile_pool(name="acts", bufs=2))
    p_t = ctx.enter_context(tc.tile_pool(name="ptrans", bufs=2, space="PSUM"))
    p_s = ctx.enter_context(tc.tile_pool(name="pint", bufs=2, space="PSUM"))
    p_x = ctx.enter_context(tc.tile_pool(name="px1", bufs=2, space="PSUM"))
    p_m = ctx.enter_context(tc.tile_pool(name="pmlp", bufs=2, space="PSUM"))

    # ---------------- load constants ----------------
    idx_sb = const.tile([104, NBLK * 32], I32)
    nc.sync.dma_start(out=idx_sb[:], in_=idx_d)
    ident = const.tile([P, P], BF16)
    nc.sync.dma_start(out=ident[:], in_=ident_d)

    def load_w(name, d, k, n, dt=BF16):
        tiles = []
        for i in range((k + P - 1) // P):
            ck = min(P, k - i * P)
            t = const.tile([ck, n], dt, name=f"{name}_{i}")
            nc.sync.dma_start(out=t[:], in_=d[i * P : i * P + ck, :])
            tiles.append(t)
        return tiles

    wd1 = load_w("wd1", wd1_d, 13, 512)
    wd2 = load_w("wd2", wd2_d, 512, 256)
    wdf = load_w("wdf", wdf_d, 256, 128)

    def load8(name, d, cols):
        t = const.tile([P, cols], FP8, name=name)
        nc.sync.dma_start(out=t[:], in_=d)
        return t

    w1f8 = load8("w1f8", w1f8_d, 6 * 1024)[:].rearrange("p (k m) -> p k m", k=6)
    w1d8 = load8("w1d8", w1d8_d, 1024)
    wt28 = load8("wt28", wt28_d, 8 * 1024)[:].rearrange("p (k m) -> p k m", k=8)
    wt38 = load8("wt38", wt38_d, 8 * 512)[:].rearrange("p (k m) -> p k m", k=8)
    wt48 = load8("wt48", wt48_d, 4 * 256)[:].rearrange("p (k m) -> p k m", k=4)
    wo8 = load8("wo8", wo8_d, 2)[:].rearrange("p (k m) -> p k m", k=2)

    def load_b(name, d, nm):
        t = const.tile([d.shape[0], nm], F32, name=name)
        nc.sync.dma_start(out=t[:], in_=d)
        return t

    bd1 = load_b("bd1", bd1_d, 4)
    bd2 = load_b("bd2", bd2_d, 2)
    bdf = load_b("bdf", bdf_d, 1)
    bdf4 = load_b("bdf4", bdf4_d, 1)
    bt1 = load_b("bt1", bt1_d, 8)
    bt2 = load_b("bt2", bt2_d, 8)
    bt3 = load_b("bt3", bt3_d, 4)
    bt4 = load_b("bt4", bt4_d, 2)
    bo = load_b("bo", bo_d, 1)

    denseT = const.tile([P, BC], BF16)          # bottom-MLP out, bf16 (for Z)
    dense8 = const.tile([P, BC], FP8)           # 4 * dense, fp8 (for L1)

    # ---------------- bottom MLP (whole core batch) ----------------
    with tc.tile_pool(name="bottom", bufs=1) as bot:
        xtb = bot.tile([13, BC], BF16)
        nc.sync.dma_start(out=xtb[:], in_=xt_d)

        h1 = [bot.tile([P, BC], BF16, name=f"h1_{m}") for m in range(4)]
        for m in range(4):
            for v in range(NV):
                pm = p_m.tile([P, VT], F32)
                nc.tensor.matmul(
                    pm[:], wd1[0][:, ts(m, P)], xtb[:, ts(v, VT)],
                    start=True, stop=True,
                )
                nc.scalar.activation(
                    h1[m][:, ts(v, VT)], pm[:], AFT.Relu, bias=bd1[:, m : m + 1]
                )

        h2 = [bot.tile([P, BC], BF16, name=f"h2_{m}") for m in range(2)]
        for m in range(2):
            for v in range(NV):
                pm = p_m.tile([P, VT], F32)
                for k in range(4):
                    nc.tensor.matmul(
                        pm[:], wd2[k][:, ts(m, P)], h1[k][:, ts(v, VT)],
                        start=(k == 0), stop=(k == 3),
                    )
                nc.scalar.activation(
                    h2[m][:, ts(v, VT)], pm[:], AFT.Relu, bias=bd2[:, m : m + 1]
                )

        for v in range(NV):
            pm = p_m.tile([P, VT], F32)
            for k in range(2):
                nc.tensor.matmul(
                    pm[:], wdf[k][:], h2[k][:, ts(v, VT)],
                    start=(k == 0), stop=(k == 1),
                )
            nc.scalar.activation(
                denseT[:, ts(v, VT)], pm[:], AFT.Relu, bias=bdf[:, 0:1]
            )
            nc.scalar.activation(
                dense8[:, ts(v, VT)], pm[:], AFT.Relu,
                bias=bdf4[:, 0:1], scale=D_SCALE,
            )

    # ---------------- fused phase I + top MLP ----------------
    x1ta_cur = None

    def top_mlp(v, x1ta_t):
        x1ta = x1ta_t[:].rearrange("p (k b) -> p k b", k=6)
        o1 = opool.tile([P, 8 * VT], FP8, name="o1")
        o2 = opool.tile([P, 8 * VT], FP8, name="o2")
        o3 = opool.tile([P, 4 * VT], FP8, name="o3")
        o4 = opool.tile([P, 2 * VT], FP8, name="o4")

        # layer 1: K = 6 pair-tiles (3 DoubleRow) + 1 dense tile
        for m in range(8):
            pm = p_m.tile([P, VT], F32)
            for j in range(3):
                nc.tensor.matmul(
                    pm[:], w1f8[:, 2 * j : 2 * j + 2, ts(m, P)],
                    x1ta[:, 2 * j : 2 * j + 2, :],
                    start=(j == 0), stop=False, perf_mode=DR,
                )
            nc.tensor.matmul(
                pm[:], w1d8[:, ts(m, P)], dense8[:, ts(v, VT)],
                start=False, stop=True,
            )
            nc.scalar.activation(
                o1[:, ts(m, VT)], pm[:], AFT.Relu,
                bias=bt1[:, m : m + 1], scale=1.0 / (S_SCALE * W1_SCALE),
            )

        def layer(out_t, in_t, w8, bias, nm, nk):
            in_v = in_t[:].rearrange("p (k b) -> p k b", k=nk)
            for m in range(nm):
                pm = p_m.tile([P, VT], F32)
                for k in range(nk // 2):
                    nc.tensor.matmul(
                        pm[:], w8[:, 2 * k : 2 * k + 2, ts(m, P)],
                        in_v[:, 2 * k : 2 * k + 2, :],
                        start=(k == 0), stop=(k == nk // 2 - 1), perf_mode=DR,
                    )
                nc.scalar.activation(
                    out_t[:, ts(m, VT)], pm[:], AFT.Relu,
                    bias=bias[:, m : m + 1], scale=1.0 / W_SCALE,
                )

        layer(o2, o1, wt28, bt2, 8, 8)
        layer(o3, o2, wt38, bt3, 4, 8)
        layer(o4, o3, wt48, bt4, 2, 4)

        pm = p_m.tile([1, VT], F32)
        o4v = o4[:].rearrange("p (k b) -> p k b", k=2)
        nc.tensor.matmul(
            pm[:], wo8[:, 0:2, :], o4v[:, 0:2, :],
            start=True, stop=True, perf_mode=DR,
        )
        zf = x1pool.tile([1, VT], F32, name="zfinal")
        nc.scalar.activation(
            zf[:], pm[:], AFT.Sigmoid, bias=bo[:], scale=1.0 / W_SCALE
        )
        nc.sync.dma_start(
            out=out_d[ts(v, VT), :].rearrange("b one -> one b"), in_=zf[:]
        )

    # gather partition p = s*26 + j (s = sample-in-group, j = feature); one
    # gather per 128-sample block covers 32 groups of 4 samples.
    for t in range(NBLK):
        if t % VBLK == 0:
            x1ta_cur = xtpool.tile([P, 6 * VT], FP8, name="x1ta")

        g = gpool.tile([104, 32 * E], BF16)
        nc.gpsimd.indirect_dma_start(
            out=g[:],
            out_offset=None,
            in_=emb,
            in_offset=bass.IndirectOffsetOnAxis(
                ap=idx_sb[:, t * 32 : (t + 1) * 32], axis=0
            ),
        )

        # transpose each group's [104, E] -> [E, 104]; zt col = q*108 + s*27 + j
        zt = zpool.tile([P, 32 * 108], BF16)
        ztv = zt[:].rearrange("p (q s j) -> p q s j", s=4, j=N_FEAT)
        for q0 in range(0, 32, 4):
            pt = p_t.tile([P, 4 * 104], BF16)
            for gq in range(4):
                nc.tensor.transpose(
                    pt[:, ts(gq, 104)],
                    g[:, ts(q0 + gq, E)],
                    ident[:104, :104],
                )
            nc.vector.tensor_copy(
                out=ztv[:, q0 : q0 + 4, :, :N_CAT],
                in_=pt[:].rearrange("p (gq s j) -> p gq s j", s=4, j=N_CAT),
            )
        # dense column at j = 26
        nc.gpsimd.tensor_copy(
            out=ztv[:, :, :, N_CAT],
            in_=denseT[:, ts(t, P)].rearrange("p (q s) -> p q s", s=4),
        )

        # interactions: 32 packed matmuls of 4 samples each; diagonal-select
        # the per-sample S blocks into spk2 [108=(s,j1), 32*27=(q,j2)] bf16
        spk2 = spool.tile([108, 32 * N_FEAT], BF16)
        s2v = spk2[:].rearrange("p (q j) -> p q j", j=N_FEAT)
        for q0 in range(0, 32, 4):
            pi = p_s.tile([108, 4 * 108], F32)
            for q in range(q0, q0 + 4):
                op = zt[:, ds(q * 108, 108)]
                nc.tensor.matmul(
                    pi[:, ts(q - q0, 108)], op, op, start=True, stop=True
                )
            piv = pi[:].rearrange("p (qq s2 j2) -> p qq s2 j2", s2=4, j2=N_FEAT)
            for s in range(4):
                src = piv[ds(s * N_FEAT, N_FEAT), :, s, :]
                dst = s2v[ds(s * N_FEAT, N_FEAT), q0 : q0 + 4, :]
                if s % 2 == 0:
                    nc.scalar.activation(dst, src, AFT.Copy)
                else:
                    nc.gpsimd.tensor_copy(out=dst, in_=src)

        # one SBUF->SBUF DMA: [(s,j1), (q,j2)] -> [(q,s) part, (j1,j2) free]
        x1s = x1pool.tile([P, 768], BF16)
        nc.any.memset(x1s[:, 729:768], 0.0)
        nc.sync.dma_start(
            out=x1s[:, :729],
            in_=spk2[:].rearrange(
                "(s j1) (q j2) -> q s j1 j2", j1=N_FEAT, j2=N_FEAT
            ),
        )

        # transpose to X1T [729(+pad), samples], cast fp8 with x128 scale
        x1ta = x1ta_cur[:].rearrange("p (k b) -> p k b", k=6)
        tt = t % VBLK
        for kh in range(2):
            px = p_x.tile([P, 3 * P], BF16)
            for k3 in range(3):
                nc.tensor.transpose(
                    px[:, ts(k3, P)], x1s[:, ts(3 * kh + k3, P)], ident[:]
                )
            nc.vector.tensor_scalar_mul(
                x1ta[:, 3 * kh : 3 * kh + 3, ts(tt, P)],
                px[:].rearrange("p (k b) -> p k b", k=3),
                S_SCALE,
            )

        if t % VBLK == VBLK - 1:
            top_mlp(t // VBLK, x1ta_cur)


# ---------------------------------------------------------------------------
# host side
# ---------------------------------------------------------------------------

_CACHE = {}


def _get_nc(cfg: Cfg):
    key = (cfg.vocab, cfg.nblk)
    if key in _CACHE:
        return _CACHE[key]
    nc = bacc.Bacc(
        "TRN2",
        target_bir_lowering=False,
        debug=False,
        enable_asserts=False,
        num_devices=N_CORES,
    )
    with tile.TileContext(nc) as tc:
        build_kernel(nc, tc, cfg)
    nc.compile()
    _CACHE[key] = nc
    return nc


def _f8(x, scale=1.0):
    f8 = mybir.dt.np(mybir.dt.float8e4)
    return np.ascontiguousarray(
        np.clip(np.asarray(x, np.float32) * scale, -224.0, 224.0).astype(f8)
    )


def _dr(w, scale):
    """[K, M] -> DoubleRow fp8 layout [128, K/128 * M]."""
    w = np.asarray(w, np.float32)
    k, m = w.shape
    assert k % P == 0
    return _f8(w.reshape(k // P, P, m).transpose(1, 0, 2).reshape(P, -1), scale)


def _prep_host(inputs, cfg: Cfg):
    """Build the per-core in_maps from full inputs."""
    bf = ml_dtypes.bfloat16
    emb = np.ascontiguousarray(
        np.asarray(inputs["emb_table"], dtype=np.float32)
        .reshape(N_CAT * cfg.vocab, E)
        .astype(bf)
    )
    cat = np.asarray(inputs["cat_idx"])
    dx = np.asarray(inputs["dense_x"], dtype=np.float32)

    iu, ju = np.triu_indices(N_FEAT, k=1)
    wt1 = np.asarray(inputs["Wt1"], dtype=np.float32)  # [479, 1024]
    w1f = np.zeros((N_FEAT, N_FEAT, 1024), dtype=np.float32)
    w1f[iu, ju] = 0.5 * wt1[: len(iu)]
    w1f[ju, iu] = 0.5 * wt1[: len(iu)]
    w1f = w1f.reshape(729, 1024)
    w1fp = np.zeros((768, 1024), dtype=np.float32)
    w1fp[:729] = w1f
    w1d = wt1[len(iu) :]  # [128, 1024]

    def b2(x, nm):  # bias [N] -> [128, nm]
        return np.ascontiguousarray(
            np.asarray(x, np.float32).reshape(nm, P).T
        )

    shared = dict(
        emb=emb,
        wd1=np.asarray(inputs["Wd1"], bf),
        wd2=np.asarray(inputs["Wd2"], bf),
        wdf=np.asarray(inputs["Wdf"], bf),
        w1f8=_dr(w1fp, W1_SCALE),
        w1d8=_f8(w1d, W1_SCALE * S_SCALE / D_SCALE),
        wt28=_dr(np.asarray(inputs["Wt2"], np.float32), W_SCALE),
        wt38=_dr(np.asarray(inputs["Wt3"], np.float32), W_SCALE),
        wt48=_dr(np.asarray(inputs["Wt4"], np.float32), W_SCALE),
        wo8=_dr(np.asarray(inputs["Wo"], np.float32), W_SCALE),
        bd1=b2(inputs["bd1"], 4),
        bd2=b2(inputs["bd2"], 2),
        bdf=b2(inputs["bdf"], 1),
        bdf4=b2(np.asarray(inputs["bdf"], np.float32) * D_SCALE, 1),
        bt1=b2(inputs["bt1"], 8),
        bt2=b2(inputs["bt2"], 8),
        bt3=b2(inputs["bt3"], 4),
        bt4=b2(inputs["bt4"], 2),
        bo=np.asarray(inputs["bo"], np.float32).reshape(1, 1),
        ident=np.eye(P, dtype=bf),
    )

    in_maps = []
    for c in range(N_CORES):
        sl = slice(c * cfg.bc, (c + 1) * cfg.bc)
        ci = cat[sl].astype(np.int64)
        rows = (np.arange(N_CAT, dtype=np.int64) * cfg.vocab)[None, :] + ci
        # idx[p=(s*26+j), t*32+q] = row of sample (t*128+4q+s), feature j
        a = rows.reshape(cfg.nblk, 32, 4, N_CAT)
        idxc = np.ascontiguousarray(
            a.transpose(2, 3, 0, 1).reshape(4 * N_CAT, cfg.nblk * 32)
        ).astype(np.int32)
        xtc = np.ascontiguousarray(dx[sl].T.astype(bf))  # [13, bc]
        in_maps.append(dict(shared, idx=idxc, xt=xtc))
    return in_maps


def run_cores(inputs, cfg: Cfg, trace=False, **kw):
    import concourse.bass_utils as bass_utils

    nc = _get_nc(cfg)
    in_maps = _prep_host(inputs, cfg)
    res = bass_utils.run_bass_kernel_spmd(
        nc, in_maps, core_ids=list(range(N_CORES)), trace=trace, **kw
    )
    out = np.concatenate([r["out"] for r in res.results], axis=0)
    return out, res


def kernel(**inputs) -> np.ndarray:
    cfg = Cfg()
    out, _ = run_cores(inputs, cfg)
    return out.astype(np.float32)
